# revision 12
# baseline (speedup 1.0000x reference)
"""DigitCaps kernel for 8 Trainium2 NeuronCores.

Math (per batch b):
    U_hat[b,d,n,j] = sum_i W[d,n,j,i] * u[b,n,i]
    A_sum[b,d,m]   = sum_n U_hat[b,d,n,:] . U_hat[b,d,m,:] / sqrt(dp)
                   = s[b,d,:] . U_hat[b,d,m,:] / sqrt(dp),  s = sum_n U_hat
    C              = softmax_d(A_sum)
    S[b,d,j]       = sum_m (B_prior[d,m] + C[b,d,m]) * U_hat[b,d,m,j]
    out            = squash(S)

The huge [B,D,N,N] similarity matrix collapses because it is immediately
summed over n - only the n-sum s of U_hat is needed.

Sharding: data-parallel over batch, 2 batches per core, W/B_prior replicated.

Per-core layout: n-tiles of 128 on partitions.
    W_sb[nt]  : [n=128, (d,j,i)=1280]   (natural, 512B-contiguous rows)
    U2[nt]    : [n=128, (b,d,j)=320]    via DVE mult(+bcast APs) + seg-reduce
    s         : ones[128,128].T @ U2    (PE partition-reduce, PSUM accum, all rows = s)
    A_sum     : DVE mult + seg-reduce over j
    softmax_d : ACT exp + DVE seg-reduce + reciprocal + mult
    S         : PE matmul lhsT=(B_prior+C) tiles, rhs=U2, PSUM accum -> [20,320]
    diag      : iota mask + mult + strided seg-reduce -> [20,16]
    squash    : small DVE/ACT ops
"""

import math
import numpy as np

import concourse.bacc as bacc
import concourse.bass as bass
import concourse.tile as tile
from concourse import mybir
from concourse.bass_utils import run_bass_kernel_spmd

F32 = mybir.dt.float32
I32 = mybir.dt.int32
AX = mybir.AxisListType
OP = mybir.AluOpType
ACTF = mybir.ActivationFunctionType

B, N, DP = 16, 1152, 8
D, DD = 10, 16
NCORES = 8
BPC = B // NCORES            # 2 batches per core
NT = N // 128                # 9 n-tiles
FW = D * DD * DP             # 1280 W free size
FU = BPC * D * DD            # 320 U2 free size
NBD = BPC * D                # 20 (b,d) pairs
EPS = 1e-7
INV_SQRT_DP = 1.0 / math.sqrt(DP)


import os

_DBG_PHASE = int(os.environ.get("KPHASE", "99"))  # dev bisect knob


def _build_kernel(tc: "tile.TileContext", out_ap, pc, W, Bp):
    nc = tc.nc
    with (
        tc.tile_pool(name="wpool", bufs=3) as wpool,
        tc.tile_pool(name="upool", bufs=3) as upool,
        tc.tile_pool(name="p2pool", bufs=2) as p2pool,
        tc.tile_pool(name="tapool", bufs=2) as tapool,
        tc.tile_pool(name="persist", bufs=1) as persist,
        tc.tile_pool(name="psum_s", bufs=1, space="PSUM") as psum_s,
        tc.tile_pool(name="psum_S2", bufs=1, space="PSUM") as psum_S2,
    ):
        ones_t = persist.tile([128, 128], F32, tag="ones")
        nc.vector.memset(ones_t[:], 1.0)

        u2_all = persist.tile([128, NT * FU], F32, tag="u2all")
        bp_all = persist.tile([128, NT * D], F32, tag="bpall")
        a_all = persist.tile([128, NT * NBD], F32, tag="aall")
        e_all = persist.tile([128, NT * NBD], F32, tag="eall")
        cb_all = persist.tile([128, NT * NBD], F32, tag="cball")

        s_ps = psum_s.tile([128, FU], F32, tag="sps")
        s_sb = persist.tile([128, FU], F32, tag="ssb")

        # ---- phase 1: load + votes U2 + running s (PE) ----
        for nt in range(NT):
            nsl = slice(nt * 128, (nt + 1) * 128)

            w_t = wpool.tile([128, FW], F32, tag="w")
            nc.sync.dma_start(w_t[:], W[:, nsl, :, :].transpose([1, 0, 2, 3]))

            u_t = upool.tile([128, BPC * DP], F32, tag="u")
            nc.sync.dma_start(u_t[:], pc[:, nsl, :].transpose([1, 0, 2]))

            nc.sync.dma_start(
                bp_all[:, nt * D:(nt + 1) * D],
                Bp[:, 0, nsl].transpose([1, 0]),
            )

            if _DBG_PHASE < 1:
                continue
            # P2[n,(b,d,j,i)] = W[n,(d,j,i)] * u[n,(b,i)] (bcast d,j)
            # ISA allows at most 3 free dims per AP -> one mult per batch b.
            p2 = p2pool.tile([128, BPC * FW], F32, tag="p2")
            w_v = w_t[:].rearrange("p (d j i) -> p d j i", d=D, j=DD, i=DP)
            for b in range(BPC):
                u_v = (
                    u_t[:, b * DP:(b + 1) * DP]
                    .unsqueeze(1)
                    .unsqueeze(1)
                    .broadcast_to([128, D, DD, DP])
                )
                p2_v = p2[:, b * FW:(b + 1) * FW].rearrange(
                    "p (d j i) -> p d j i", d=D, j=DD, i=DP
                )
                nc.vector.tensor_tensor(p2_v, w_v, u_v, OP.mult)

            # U2[n,(b,d,j)] = sum_i P2
            nc.vector.tensor_reduce(
                u2_all[:, nt * FU:(nt + 1) * FU],
                p2[:].rearrange("p (g i) -> p g i", g=FU, i=DP),
                AX.X,
                OP.add,
            )

            if _DBG_PHASE < 2:
                continue
            # s accumulation: every row of s_ps ends up = sum_n U2[n, :]
            nc.tensor.matmul(
                s_ps[:],
                ones_t[:],
                u2_all[:, nt * FU:(nt + 1) * FU],
                start=(nt == 0),
                stop=(nt == NT - 1),
            )

        def _dbg_out(src):
            nc.sync.dma_start(out_ap.rearrange("b d j -> (b d) j"), src)

        if _DBG_PHASE < 2:
            _dbg_out(u2_all[:NBD, :DD])
            return

        # s scaled by 1/sqrt(dp) (folds the A/sqrt(dp) scaling)
        nc.scalar.mul(s_sb[:], s_ps[:], INV_SQRT_DP)
        if _DBG_PHASE < 3:
            _dbg_out(s_sb[:NBD, :DD])
            return

        # ---- phase 2: A_sum[b,d,m] = sum_j U2 * s ----
        for nt in range(NT if _DBG_PHASE >= 3 else 0):
            ta = tapool.tile([128, FU], F32, tag="ta")
            nc.vector.tensor_tensor(
                ta[:], u2_all[:, nt * FU:(nt + 1) * FU], s_sb[:], OP.mult
            )
            nc.vector.tensor_reduce(
                a_all[:, nt * NBD:(nt + 1) * NBD],
                ta[:].rearrange("p (g j) -> p g j", g=NBD, j=DD),
                AX.X,
                OP.add,
            )

        if _DBG_PHASE < 4:
            _dbg_out(a_all[:NBD, :DD])
            return

        # ---- phase 3: softmax over d + B_prior ----
        nc.scalar.activation(e_all[:], a_all[:], ACTF.Exp)
        z_t = persist.tile([128, NT * BPC], F32, tag="z")
        nc.vector.tensor_reduce(
            z_t[:],
            e_all[:].rearrange("p (g d) -> p g d", g=NT * BPC, d=D),
            AX.X,
            OP.add,
        )
        zr_t = persist.tile([128, NT * BPC], F32, tag="zr")
        nc.vector.reciprocal(zr_t[:], z_t[:])

        e_v = e_all[:].rearrange("p (g d) -> p g d", g=NT * BPC, d=D)
        zr_v = zr_t[:].unsqueeze(2).broadcast_to([128, NT * BPC, D])
        c_v = cb_all[:].rearrange("p (g d) -> p g d", g=NT * BPC, d=D)
        nc.vector.tensor_tensor(c_v, e_v, zr_v, OP.mult)

        # += B_prior (broadcast over b)
        cb_v = cb_all[:].rearrange("p (t b d) -> p t b d", t=NT, b=BPC, d=D)
        bp_v = (
            bp_all[:]
            .rearrange("p (t d) -> p t d", t=NT, d=D)
            .unsqueeze(2)
            .broadcast_to([128, NT, BPC, D])
        )
        nc.vector.tensor_tensor(cb_v, cb_v, bp_v, OP.add)

        if _DBG_PHASE < 5:
            _dbg_out(cb_all[:NBD, :DD])
            return

        # ---- phase 4: S[bd, (b',d',j)] = sum_n cb * U2 ----
        S2_ps = psum_S2.tile([NBD, FU], F32, tag="S2")
        for nt in range(NT):
            nc.tensor.matmul(
                S2_ps[:],
                cb_all[:, nt * NBD:(nt + 1) * NBD],
                u2_all[:, nt * FU:(nt + 1) * FU],
                start=(nt == 0),
                stop=(nt == NT - 1),
            )

        if _DBG_PHASE < 6:
            sdbg = persist.tile([NBD, DD], F32, tag="sdbg")
            nc.vector.tensor_copy(sdbg[:], S2_ps[:NBD, :DD])
            _dbg_out(sdbg[:])
            return

        # ---- phase 5: extract diagonal (b,d)=(b',d') via iota mask ----
        iota_t = persist.tile([NBD, FU], I32, tag="iota")
        nc.gpsimd.iota(
            iota_t[:], pattern=[[1, NBD], [0, DD]], base=0, channel_multiplier=-1
        )
        mask_t = persist.tile([NBD, FU], F32, tag="mask")
        nc.vector.tensor_scalar(mask_t[:], iota_t[:], 0, None, OP.is_equal)

        sm_t = persist.tile([NBD, FU], F32, tag="sm")
        nc.vector.tensor_tensor(sm_t[:], S2_ps[:], mask_t[:], OP.mult)
        s_diag = persist.tile([NBD, DD], F32, tag="sdiag")
        nc.vector.tensor_reduce(
            s_diag[:],
            sm_t[:].rearrange("p (g j) -> p j g", g=NBD, j=DD),
            AX.X,
            OP.add,
        )

        if _DBG_PHASE < 7:
            _dbg_out(s_diag[:])
            return

        # ---- phase 6: squash ----
        def _dbg_col(src):
            sdbg = persist.tile([NBD, DD], F32, tag="sdbg")
            nc.vector.tensor_copy(sdbg[:, :1], src)
            _dbg_out(sdbg[:])

        ss_t = persist.tile([NBD, DD], F32, tag="ss")
        nrm2 = persist.tile([NBD, 1], F32, tag="nrm2")
        nc.vector.tensor_tensor(ss_t[:], s_diag[:], s_diag[:], OP.mult)
        nc.vector.tensor_reduce(nrm2[:], ss_t[:], AX.X, OP.add)
        if _DBG_PHASE < 8:
            _dbg_col(nrm2[:])
            return
        nrm = persist.tile([NBD, 1], F32, tag="nrm")
        nc.scalar.sqrt(nrm[:], nrm2[:])
        if _DBG_PHASE < 9:
            _dbg_col(nrm[:])
            return
        en = persist.tile([NBD, 1], F32, tag="en")
        nc.scalar.activation(en[:], nrm[:], ACTF.Exp)
        if _DBG_PHASE < 10:
            _dbg_col(en[:])
            return
        en_eps = persist.tile([NBD, 1], F32, tag="eneps")
        nc.vector.tensor_scalar(en_eps[:], en[:], EPS, None, OP.add)
        r1 = persist.tile([NBD, 1], F32, tag="r1")
        nc.vector.reciprocal(r1[:], en_eps[:])
        if _DBG_PHASE < 11:
            _dbg_col(r1[:])
            return
        coef = persist.tile([NBD, 1], F32, tag="coef")
        nc.vector.tensor_scalar(coef[:], r1[:], -1.0, 1.0, OP.mult, OP.add)
        nrm_eps = persist.tile([NBD, 1], F32, tag="nrmeps")
        nc.vector.tensor_scalar(nrm_eps[:], nrm[:], EPS, None, OP.add)
        r2 = persist.tile([NBD, 1], F32, tag="r2")
        nc.vector.reciprocal(r2[:], nrm_eps[:])
        if _DBG_PHASE < 12:
            _dbg_col(r2[:])
            return
        fac = persist.tile([NBD, 1], F32, tag="fac")
        nc.vector.tensor_tensor(fac[:], coef[:], r2[:], OP.mult)

        res_t = persist.tile([NBD, DD], F32, tag="res")
        nc.vector.tensor_scalar(res_t[:], s_diag[:], fac[:], None, OP.mult)

        nc.sync.dma_start(out_ap.rearrange("b d j -> (b d) j"), res_t[:])


_CACHE: dict = {}


def _get_nc():
    if "nc" not in _CACHE:
        nc = bacc.Bacc(
            "TRN2", target_bir_lowering=False, debug=False, num_devices=NCORES
        )
        pc = nc.dram_tensor("primary_caps", [BPC, N, DP], F32, kind="ExternalInput").ap()
        W = nc.dram_tensor("W", [D, N, DD, DP], F32, kind="ExternalInput").ap()
        Bp = nc.dram_tensor("B_prior", [D, 1, N], F32, kind="ExternalInput").ap()
        out = nc.dram_tensor("out", [BPC, D, DD], F32, kind="ExternalOutput").ap()
        with tile.TileContext(nc) as tc:
            _build_kernel(tc, out, pc, W, Bp)
        nc.compile()
        _CACHE["nc"] = nc
    return _CACHE["nc"]


def _run(primary_caps, W, B_prior, trace=False, **kw):
    nc = _get_nc()
    in_maps = [
        {
            "primary_caps": np.ascontiguousarray(
                primary_caps[c * BPC:(c + 1) * BPC]
            ).astype(np.float32),
            "W": np.asarray(W, dtype=np.float32),
            "B_prior": np.asarray(B_prior, dtype=np.float32),
        }
        for c in range(NCORES)
    ]
    res = run_bass_kernel_spmd(nc, in_maps, list(range(NCORES)), trace=trace, **kw)
    out = np.concatenate([res.results[c]["out"] for c in range(NCORES)], axis=0)
    return out.astype(np.float32), res


def kernel(primary_caps, W, B_prior):
    out, _ = _run(primary_caps, W, B_prior, trace=False)
    return out


# revision 13
# speedup vs baseline: 1.4899x; 1.4899x over previous
"""DigitCaps kernel for 8 Trainium2 NeuronCores.

Math (per batch b):
    U_hat[b,d,n,j] = sum_i W[d,n,j,i] * u[b,n,i]
    A_sum[b,d,m]   = sum_n U_hat[b,d,n,:] . U_hat[b,d,m,:] / sqrt(dp)
                   = s[b,d,:] . U_hat[b,d,m,:] / sqrt(dp),  s = sum_n U_hat
    C              = softmax_d(A_sum)
    S[b,d,j]       = sum_m (B_prior[d,m] + C[b,d,m]) * U_hat[b,d,m,j]
    out            = squash(S)

The huge [B,D,N,N] similarity matrix collapses because it is immediately
summed over n - only the n-sum s of U_hat is needed.

Sharding: data-parallel over batch, 2 batches per core, W/B_prior replicated.

Per-core layout: n-tiles of 128 on partitions.
    W_sb[nt]  : [n=128, (d,j,i)=1280]   (natural, 512B-contiguous rows)
    U2[nt]    : [n=128, (b,d,j)=320]    DVE multiply-accumulate chain over i
    s         : ones[128,128].T @ U2    (PE partition-reduce, PSUM accum; all
                                         rows of the PSUM tile equal s)
    then per n-tile (pipelined): A_sum (DVE) -> exp (ACT, scale=1/sqrt(dp))
    -> softmax_d norm (DVE) -> +B_prior -> S matmul (PE, PSUM accum)
    diagonal extract via iota mask, squash with small DVE/ACT ops.
"""

import math
import numpy as np

import concourse.bacc as bacc
import concourse.bass as bass
import concourse.tile as tile
from concourse import mybir
from concourse.bass_utils import run_bass_kernel_spmd

F32 = mybir.dt.float32
I32 = mybir.dt.int32
AX = mybir.AxisListType
OP = mybir.AluOpType
ACTF = mybir.ActivationFunctionType

B, N, DP = 16, 1152, 8
D, DD = 10, 16
NCORES = 8
BPC = B // NCORES            # 2 batches per core
NT = N // 128                # 9 n-tiles
FW = D * DD * DP             # 1280 W free size
FD = D * DD                  # 160 per-batch U2 free size
FU = BPC * FD                # 320 U2 free size
NBD = BPC * D                # 20 (b,d) pairs
EPS = 1e-7
INV_SQRT_DP = 1.0 / math.sqrt(DP)


def _build_kernel(tc: "tile.TileContext", out_ap, pc, W, Bp):
    nc = tc.nc
    with (
        tc.tile_pool(name="wpool", bufs=3) as wpool,
        tc.tile_pool(name="upool", bufs=3) as upool,
        tc.tile_pool(name="tapool", bufs=2) as tapool,
        tc.tile_pool(name="smpool", bufs=2) as smpool,
        tc.tile_pool(name="persist", bufs=1) as persist,
        tc.tile_pool(name="psum_s", bufs=1, space="PSUM") as psum_s,
        tc.tile_pool(name="psum_S2", bufs=1, space="PSUM") as psum_S2,
    ):
        ones_t = persist.tile([128, 128], F32, tag="ones")
        nc.vector.memset(ones_t[:], 1.0)

        u2_all = persist.tile([128, NT * FU], F32, tag="u2all")
        bp_all = persist.tile([128, NT * D], F32, tag="bpall")
        cb_all = persist.tile([128, NT * NBD], F32, tag="cball")
        e_all = persist.tile([128, NT * NBD], F32, tag="eall")
        z_all = persist.tile([128, NT * BPC], F32, tag="zall")
        zr_all = persist.tile([128, NT * BPC], F32, tag="zrall")

        s_ps = psum_s.tile([128, FU], F32, tag="sps")

        # ---- phase 1: load; U2 votes via i-chain; running s on PE ----
        for nt in range(NT):
            nsl = slice(nt * 128, (nt + 1) * 128)

            w_t = wpool.tile([128, FW], F32, tag="w")
            nc.sync.dma_start(w_t[:], W[:, nsl, :, :].transpose([1, 0, 2, 3]))

            u_t = upool.tile([128, BPC * DP], F32, tag="u")
            nc.sync.dma_start(u_t[:], pc[:, nsl, :].transpose([1, 0, 2]))

            nc.sync.dma_start(
                bp_all[:, nt * D:(nt + 1) * D],
                Bp[:, 0, nsl].transpose([1, 0]),
            )

            # U2[n,(b,d,j)] += W[n,(d,j,i)] * u[n,(b,i)]  accumulated over i
            w_3 = w_t[:].rearrange("p (dj i) -> p dj i", dj=FD, i=DP)
            for b in range(BPC):
                u2_sl = u2_all[:, nt * FU + b * FD: nt * FU + (b + 1) * FD]
                nc.vector.tensor_scalar(
                    u2_sl, w_3[:, :, 0], u_t[:, b * DP: b * DP + 1], None, OP.mult
                )
                for i in range(1, DP):
                    nc.vector.scalar_tensor_tensor(
                        u2_sl,
                        w_3[:, :, i],
                        u_t[:, b * DP + i: b * DP + i + 1],
                        u2_sl,
                        OP.mult,
                        OP.add,
                    )

            # s accumulation: every row of s_ps ends up = sum_n U2[n, :]
            nc.tensor.matmul(
                s_ps[:],
                ones_t[:],
                u2_all[:, nt * FU:(nt + 1) * FU],
                start=(nt == 0),
                stop=(nt == NT - 1),
            )

        # ---- phase 2 (pipelined per n-tile): A_sum -> softmax_d -> +B_prior
        #      -> S matmul ----
        S2_ps = psum_S2.tile([NBD, FU], F32, tag="S2")
        for nt in range(NT):
            u2_sl = u2_all[:, nt * FU:(nt + 1) * FU]
            # TA = U2 * s  (s read straight from PSUM; all rows identical)
            ta = tapool.tile([128, FU], F32, tag="ta")
            nc.vector.tensor_tensor(ta[:], u2_sl, s_ps[:], OP.mult)
            # A[n,(b,d)] = sum_j TA
            a_sl = e_all[:, nt * NBD:(nt + 1) * NBD]  # staging (overwritten by exp)
            nc.vector.tensor_reduce(
                a_sl,
                ta[:].rearrange("p (g j) -> p g j", g=NBD, j=DD),
                AX.X,
                OP.add,
            )
            # E = exp(A / sqrt(dp))
            nc.scalar.activation(a_sl, a_sl, ACTF.Exp, scale=INV_SQRT_DP)
            # z[(b)] = sum_d E ; zr = 1/z
            z_sl = z_all[:, nt * BPC:(nt + 1) * BPC]
            zr_sl = zr_all[:, nt * BPC:(nt + 1) * BPC]
            nc.vector.tensor_reduce(
                z_sl,
                a_sl.rearrange("p (b d) -> p b d", b=BPC, d=D),
                AX.X,
                OP.add,
            )
            nc.vector.reciprocal(zr_sl, z_sl)
            # cb = E * zr + B_prior
            cb_sl = cb_all[:, nt * NBD:(nt + 1) * NBD]
            nc.vector.tensor_tensor(
                cb_sl.rearrange("p (b d) -> p b d", b=BPC, d=D),
                a_sl.rearrange("p (b d) -> p b d", b=BPC, d=D),
                zr_sl.unsqueeze(2).broadcast_to([128, BPC, D]),
                OP.mult,
            )
            nc.vector.tensor_tensor(
                cb_sl.rearrange("p (b d) -> p b d", b=BPC, d=D),
                cb_sl.rearrange("p (b d) -> p b d", b=BPC, d=D),
                bp_all[:, nt * D:(nt + 1) * D]
                .unsqueeze(1)
                .broadcast_to([128, BPC, D]),
                OP.add,
            )
            # S2 += cb.T @ U2
            nc.tensor.matmul(
                S2_ps[:],
                cb_sl,
                u2_sl,
                start=(nt == 0),
                stop=(nt == NT - 1),
            )

        # ---- phase 3: extract diagonal (b,d)=(b',d') via iota mask ----
        iota_t = persist.tile([NBD, FU], I32, tag="iota")
        nc.gpsimd.iota(
            iota_t[:], pattern=[[1, NBD], [0, DD]], base=0, channel_multiplier=-1
        )
        mask_t = persist.tile([NBD, FU], F32, tag="mask")
        nc.vector.tensor_scalar(mask_t[:], iota_t[:], 0, None, OP.is_equal)

        sm_t = smpool.tile([NBD, FU], F32, tag="sm")
        nc.vector.tensor_tensor(sm_t[:], S2_ps[:], mask_t[:], OP.mult)
        s_diag = persist.tile([NBD, DD], F32, tag="sdiag")
        nc.vector.tensor_reduce(
            s_diag[:],
            sm_t[:].rearrange("p (g j) -> p j g", g=NBD, j=DD),
            AX.X,
            OP.add,
        )

        # ---- phase 4: squash ----
        ss_t = persist.tile([NBD, DD], F32, tag="ss")
        nrm2 = persist.tile([NBD, 1], F32, tag="nrm2")
        nc.vector.tensor_tensor(ss_t[:], s_diag[:], s_diag[:], OP.mult)
        nc.vector.tensor_reduce(nrm2[:], ss_t[:], AX.X, OP.add)
        nrm = persist.tile([NBD, 1], F32, tag="nrm")
        nc.scalar.sqrt(nrm[:], nrm2[:])
        en = persist.tile([NBD, 1], F32, tag="en")
        nc.scalar.activation(en[:], nrm[:], ACTF.Exp)
        en_eps = persist.tile([NBD, 1], F32, tag="eneps")
        nc.vector.tensor_scalar(en_eps[:], en[:], EPS, None, OP.add)
        r1 = persist.tile([NBD, 1], F32, tag="r1")
        nc.vector.reciprocal(r1[:], en_eps[:])
        coef = persist.tile([NBD, 1], F32, tag="coef")
        nc.vector.tensor_scalar(coef[:], r1[:], -1.0, 1.0, OP.mult, OP.add)
        nrm_eps = persist.tile([NBD, 1], F32, tag="nrmeps")
        nc.vector.tensor_scalar(nrm_eps[:], nrm[:], EPS, None, OP.add)
        r2 = persist.tile([NBD, 1], F32, tag="r2")
        nc.vector.reciprocal(r2[:], nrm_eps[:])
        fac = persist.tile([NBD, 1], F32, tag="fac")
        nc.vector.tensor_tensor(fac[:], coef[:], r2[:], OP.mult)

        res_t = persist.tile([NBD, DD], F32, tag="res")
        nc.vector.tensor_scalar(res_t[:], s_diag[:], fac[:], None, OP.mult)

        nc.sync.dma_start(out_ap.rearrange("b d j -> (b d) j"), res_t[:])


_CACHE: dict = {}


def _get_nc():
    if "nc" not in _CACHE:
        nc = bacc.Bacc(
            "TRN2", target_bir_lowering=False, debug=False, num_devices=NCORES
        )
        pc = nc.dram_tensor("primary_caps", [BPC, N, DP], F32, kind="ExternalInput").ap()
        W = nc.dram_tensor("W", [D, N, DD, DP], F32, kind="ExternalInput").ap()
        Bp = nc.dram_tensor("B_prior", [D, 1, N], F32, kind="ExternalInput").ap()
        out = nc.dram_tensor("out", [BPC, D, DD], F32, kind="ExternalOutput").ap()
        with tile.TileContext(nc) as tc:
            _build_kernel(tc, out, pc, W, Bp)
        nc.compile()
        _CACHE["nc"] = nc
    return _CACHE["nc"]


def _run(primary_caps, W, B_prior, trace=False, **kw):
    nc = _get_nc()
    in_maps = [
        {
            "primary_caps": np.ascontiguousarray(
                primary_caps[c * BPC:(c + 1) * BPC]
            ).astype(np.float32),
            "W": np.asarray(W, dtype=np.float32),
            "B_prior": np.asarray(B_prior, dtype=np.float32),
        }
        for c in range(NCORES)
    ]
    res = run_bass_kernel_spmd(nc, in_maps, list(range(NCORES)), trace=trace, **kw)
    out = np.concatenate([res.results[c]["out"] for c in range(NCORES)], axis=0)
    return out.astype(np.float32), res


def kernel(primary_caps, W, B_prior):
    out, _ = _run(primary_caps, W, B_prior, trace=False)
    return out


# revision 34
# speedup vs baseline: 1.8535x; 1.2440x over previous
"""DigitCaps kernel for 8 Trainium2 NeuronCores.

Math (per batch b):
    U_hat[b,d,n,j] = sum_i W[d,n,j,i] * u[b,n,i]
    A_sum[b,d,m]   = sum_n U_hat[b,d,n,:] . U_hat[b,d,m,:] / sqrt(dp)
                   = s[b,d,:] . U_hat[b,d,m,:] / sqrt(dp),  s = sum_n U_hat
    C              = softmax_d(A_sum)
    S[b,d,j]       = sum_m (B_prior[d,m] + C[b,d,m]) * U_hat[b,d,m,j]
    out            = squash(S)

The huge [B,D,N,N] similarity matrix collapses because it is immediately
summed over n - only the n-sum s of U_hat is needed.

Sharding: data-parallel over batch, 2 batches per core, W/B_prior replicated.

Per-core layout: n-tiles of 128 on partitions.
    W_sb[nt]  : [n=128, (d,j,i)=1280]   (natural, 512B-contiguous rows)
    U2[nt]    : [n=128, (b,d,j)=320]    DVE multiply-accumulate chain over i
    s         : ones[128,128].T @ U2    (PE partition-reduce, PSUM accum; all
                                         rows of the PSUM tile equal s)
    then per n-tile (pipelined): A_sum (DVE) -> exp (ACT, scale=1/sqrt(dp))
    -> softmax_d norm (DVE) -> +B_prior -> S matmul (PE, PSUM accum)
    diagonal extract via iota mask, squash with small DVE/ACT ops.
"""

import math
import numpy as np

import concourse.bacc as bacc
import concourse.bass as bass
import concourse.tile as tile
from concourse import mybir
from concourse.bass_utils import run_bass_kernel_spmd

F32 = mybir.dt.float32
I32 = mybir.dt.int32
AX = mybir.AxisListType
OP = mybir.AluOpType
ACTF = mybir.ActivationFunctionType

B, N, DP = 16, 1152, 8
D, DD = 10, 16
NCORES = 8
BPC = B // NCORES            # 2 batches per core
NT = N // 128                # 9 n-tiles
FW = D * DD * DP             # 1280 W free size
FD = D * DD                  # 160 per-batch U2 free size
FU = BPC * FD                # 320 U2 free size
NBD = BPC * D                # 20 (b,d) pairs
EPS = 1e-7
INV_SQRT_DP = 1.0 / math.sqrt(DP)


def _build_kernel(tc: "tile.TileContext", out_ap, pc, W, Bp):
    nc = tc.nc
    with (
        tc.tile_pool(name="wpool", bufs=NT) as wpool,
        tc.tile_pool(name="upool", bufs=NT) as upool,
        tc.tile_pool(name="tapool", bufs=2) as tapool,
        tc.tile_pool(name="ppool", bufs=2) as ppool,
        tc.tile_pool(name="smpool", bufs=2) as smpool,
        tc.tile_pool(name="persist", bufs=1) as persist,
        tc.tile_pool(name="psum_s", bufs=1, space="PSUM") as psum_s,
        tc.tile_pool(name="psum_S2", bufs=1, space="PSUM") as psum_S2,
    ):
        BF16 = mybir.dt.bfloat16
        ones_t = persist.tile([128, 128], BF16, tag="ones")
        nc.vector.memset(ones_t[:], 1.0)

        u2_all = persist.tile([128, NT * FU], F32, tag="u2all")
        u2bf_all = persist.tile([128, NT * FU], BF16, tag="u2bfall")
        cbbf_all = persist.tile([128, NT * NBD], BF16, tag="cbbfall")
        bp_all = persist.tile([128, NT * D], F32, tag="bpall")
        cb_all = persist.tile([128, NT * NBD], F32, tag="cball")
        e_all = persist.tile([128, NT * NBD], F32, tag="eall")
        z_all = persist.tile([128, NT * BPC], F32, tag="zall")
        zr_all = persist.tile([128, NT * BPC], F32, tag="zrall")

        s_ps = psum_s.tile([128, FU], F32, tag="sps")

        # preload the Exp ACT table while ACT is idle (hides the ~1.3us
        # table load that would otherwise land in the phase-2 critical path)
        warm_t = persist.tile([1, 1], F32, tag="warm")
        nc.vector.memset(warm_t[:], 0.0)
        nc.scalar.activation(warm_t[:], warm_t[:], ACTF.Exp)

        # ---- phase 1: load; U2 votes via i-chain; running s on PE ----
        ACT_TILES = ()  # these tiles route products via ACT + GpSimd
        # (nt, b) half-chains routed to GpSimd (mult + tree, all Pool-legal)
        POOL_HALVES = {(1, 1), (3, 1), (5, 1), (7, 1)}
        for nt in range(NT):
            nsl = slice(nt * 128, (nt + 1) * 128)

            w_t = wpool.tile([128, FW], F32, tag="w")
            nc.sync.dma_start(w_t[:], W[:, nsl, :, :].transpose([1, 0, 2, 3]))

            u_t = upool.tile([128, BPC * DP], F32, tag="u")
            nc.sync.dma_start(u_t[:], pc[:, nsl, :].transpose([1, 0, 2]))

            # U2[n,(b,d,j)] += W[n,(d,j,i)] * u[n,(b,i)]  accumulated over i.
            # TensorScalarPtr is DVE-only on trn2 (walrus rejects it on Pool),
            # so offload tiles via ACT products + GpSimd tree-reduce instead.
            w_3 = w_t[:].rearrange("p (dj i) -> p dj i", dj=FD, i=DP)
            if nt in ACT_TILES:
                # products P[n,(b,dj,i)] on ACT (Copy with per-partition
                # scale), then i-tree-reduce on GpSimd
                pp = ppool.tile([128, BPC * FW], F32, tag="pp")
                pp_v = pp[:].rearrange(
                    "p (b dj i) -> p b dj i", b=BPC, dj=FD, i=DP
                )
                for b in range(BPC):
                    for i in range(DP):
                        nc.scalar.activation(
                            pp_v[:, b, :, i],
                            w_3[:, :, i],
                            ACTF.Copy,
                            scale=u_t[:, b * DP + i: b * DP + i + 1],
                        )
                t1 = ppool.tile([128, BPC * FD * 4], F32, tag="t1")
                t1_v = t1[:].rearrange("p (g i) -> p g i", g=BPC * FD, i=4)
                pp_g = pp[:].rearrange("p (g i) -> p g i", g=BPC * FD, i=DP)
                nc.gpsimd.tensor_tensor(
                    t1_v, pp_g[:, :, 0:4], pp_g[:, :, 4:8], OP.add
                )
                t2 = ppool.tile([128, BPC * FD * 2], F32, tag="t2")
                t2_v = t2[:].rearrange("p (g i) -> p g i", g=BPC * FD, i=2)
                nc.gpsimd.tensor_tensor(
                    t2_v, t1_v[:, :, 0:2], t1_v[:, :, 2:4], OP.add
                )
                nc.gpsimd.tensor_tensor(
                    u2_all[:, nt * FU:(nt + 1) * FU].rearrange(
                        "p (g i) -> p g i", g=BPC * FD, i=1
                    ),
                    t2_v[:, :, 0:1],
                    t2_v[:, :, 1:2],
                    OP.add,
                )
            else:
                for b in range(BPC):
                    u2_sl = u2_all[:, nt * FU + b * FD: nt * FU + (b + 1) * FD]
                    if (nt, b) in POOL_HALVES:
                        # GpSimd route: one big mult + 3 tree-adds over i
                        pp = ppool.tile([128, FW], F32, tag="pp")
                        pp_v = pp[:].rearrange("p (g i) -> p g i", g=FD, i=DP)
                        u_bc = (
                            u_t[:, b * DP:(b + 1) * DP]
                            .unsqueeze(1)
                            .broadcast_to([128, FD, DP])
                        )
                        nc.gpsimd.tensor_tensor(pp_v, w_3, u_bc, OP.mult)
                        t1 = ppool.tile([128, FD * 4], F32, tag="t1")
                        t1_v = t1[:].rearrange("p (g i) -> p g i", g=FD, i=4)
                        nc.gpsimd.tensor_tensor(
                            t1_v, pp_v[:, :, 0:4], pp_v[:, :, 4:8], OP.add
                        )
                        t2 = ppool.tile([128, FD * 2], F32, tag="t2")
                        t2_v = t2[:].rearrange("p (g i) -> p g i", g=FD, i=2)
                        nc.gpsimd.tensor_tensor(
                            t2_v, t1_v[:, :, 0:2], t1_v[:, :, 2:4], OP.add
                        )
                        nc.gpsimd.tensor_tensor(
                            u2_sl.rearrange("p (g i) -> p g i", g=FD, i=1),
                            t2_v[:, :, 0:1],
                            t2_v[:, :, 1:2],
                            OP.add,
                        )
                        continue
                    nc.vector.tensor_scalar(
                        u2_sl, w_3[:, :, 0], u_t[:, b * DP: b * DP + 1], None, OP.mult
                    )
                    for i in range(1, DP):
                        nc.vector.scalar_tensor_tensor(
                            u2_sl,
                            w_3[:, :, i],
                            u_t[:, b * DP + i: b * DP + i + 1],
                            u2_sl,
                            OP.mult,
                            OP.add,
                        )

            # bf16 shadow copy of U2 for the PE matmuls (ACT is idle)
            u2bf_sl = u2bf_all[:, nt * FU:(nt + 1) * FU]
            nc.scalar.copy(u2bf_sl, u2_all[:, nt * FU:(nt + 1) * FU])

            # s accumulation: every row of s_ps ends up = sum_n U2[n, :]
            nc.tensor.matmul(
                s_ps[:],
                ones_t[:],
                u2bf_sl,
                start=(nt == 0),
                stop=(nt == NT - 1),
            )

        # B_prior loads only matter in phase 2 - keep them off the W/u stream
        for nt in range(NT):
            nc.sync.dma_start(
                bp_all[:, nt * D:(nt + 1) * D],
                Bp[:, 0, nt * 128:(nt + 1) * 128].transpose([1, 0]),
            )

        # ---- phase 2 (pipelined per n-tile): A_sum -> softmax_d -> +B_prior
        #      -> S matmul ----
        # s copy to SBUF so GpSimd (no PSUM access) can read it
        s_sb = persist.tile([128, FU], F32, tag="ssb")
        nc.scalar.copy(s_sb[:], s_ps[:])

        S2_ps = psum_S2.tile([NBD, FU], F32, tag="S2")
        POOL_TILES = (3, 4, 5, 6, 7, 8)  # TA on GpSimd for these n-tiles
        for nt in range(NT):
            u2_sl = u2_all[:, nt * FU:(nt + 1) * FU]
            a_sl = e_all[:, nt * NBD:(nt + 1) * NBD]  # staging (overwritten by exp)
            ta = tapool.tile([128, FU], F32, tag="ta")
            if nt in POOL_TILES:
                nc.gpsimd.tensor_tensor(ta[:], u2_sl, s_sb[:], OP.mult)
            else:
                nc.vector.tensor_tensor(ta[:], u2_sl, s_ps[:], OP.mult)
            nc.vector.tensor_reduce(
                a_sl,
                ta[:].rearrange("p (g j) -> p g j", g=NBD, j=DD),
                AX.X,
                OP.add,
            )
            # E = exp(A / sqrt(dp))
            nc.scalar.activation(a_sl, a_sl, ACTF.Exp, scale=INV_SQRT_DP)
            # z[(b)] = sum_d E ; zr = 1/z
            z_sl = z_all[:, nt * BPC:(nt + 1) * BPC]
            zr_sl = zr_all[:, nt * BPC:(nt + 1) * BPC]
            nc.vector.tensor_reduce(
                z_sl,
                a_sl.rearrange("p (b d) -> p b d", b=BPC, d=D),
                AX.X,
                OP.add,
            )
            nc.vector.reciprocal(zr_sl, z_sl)
            # cb = E * zr + B_prior, written directly as bf16 for the matmul
            cbbf_sl = cbbf_all[:, nt * NBD:(nt + 1) * NBD]
            for b in range(BPC):
                nc.vector.scalar_tensor_tensor(
                    cbbf_sl[:, b * D:(b + 1) * D],
                    a_sl[:, b * D:(b + 1) * D],
                    zr_sl[:, b: b + 1],
                    bp_all[:, nt * D:(nt + 1) * D],
                    OP.mult,
                    OP.add,
                )
            # S2 += cb.T @ U2 (bf16 operands, fp32 PSUM accumulate)
            nc.tensor.matmul(
                S2_ps[:],
                cbbf_sl,
                u2bf_all[:, nt * FU:(nt + 1) * FU],
                start=(nt == 0),
                stop=(nt == NT - 1),
            )

        # ---- phase 3: extract diagonal (b,d)=(b',d') via iota mask ----
        iota_t = persist.tile([NBD, FU], I32, tag="iota")
        nc.gpsimd.iota(
            iota_t[:], pattern=[[1, NBD], [0, DD]], base=0, channel_multiplier=-1
        )
        mask_t = persist.tile([NBD, FU], F32, tag="mask")
        nc.vector.tensor_scalar(mask_t[:], iota_t[:], 0, None, OP.is_equal)

        sm_t = smpool.tile([NBD, FU], F32, tag="sm")
        nc.vector.tensor_tensor(sm_t[:], S2_ps[:], mask_t[:], OP.mult)
        s_diag = persist.tile([NBD, DD], F32, tag="sdiag")
        nc.vector.tensor_reduce(
            s_diag[:],
            sm_t[:].rearrange("p (g j) -> p j g", g=NBD, j=DD),
            AX.X,
            OP.add,
        )

        # ---- phase 4: squash ----
        ss_t = persist.tile([NBD, DD], F32, tag="ss")
        nrm2 = persist.tile([NBD, 1], F32, tag="nrm2")
        nc.vector.tensor_tensor(ss_t[:], s_diag[:], s_diag[:], OP.mult)
        nc.vector.tensor_reduce(nrm2[:], ss_t[:], AX.X, OP.add)
        # norm via DVE Newton sqrt (bit-hack seed + 2 iterations) - keeps the
        # Exp ACT table resident (no sqrt/exp table reload in the tail)
        nrm = persist.tile([NBD, 1], F32, tag="nrm")
        seed_i = persist.tile([NBD, 1], I32, tag="seedi")
        nc.vector.tensor_scalar(
            seed_i[:], nrm2[:].bitcast(I32), 1, None, OP.logical_shift_right
        )
        nc.vector.tensor_scalar(seed_i[:], seed_i[:], 0x1FBD1DF5, None, OP.add)
        nc.vector.tensor_copy(nrm[:], seed_i[:].bitcast(F32))
        nwr = persist.tile([NBD, 1], F32, tag="nwr")
        nwt = persist.tile([NBD, 1], F32, tag="nwt")
        for _ in range(2):
            nc.vector.reciprocal(nwr[:], nrm[:])
            nc.vector.tensor_tensor(nwt[:], nrm2[:], nwr[:], OP.mult)
            nc.vector.tensor_tensor(nrm[:], nrm[:], nwt[:], OP.add)
            nc.vector.tensor_scalar(nrm[:], nrm[:], 0.5, None, OP.mult)
        en = persist.tile([NBD, 1], F32, tag="en")
        nc.scalar.activation(en[:], nrm[:], ACTF.Exp)
        en_eps = persist.tile([NBD, 1], F32, tag="eneps")
        nc.vector.tensor_scalar(en_eps[:], en[:], EPS, None, OP.add)
        r1 = persist.tile([NBD, 1], F32, tag="r1")
        nc.vector.reciprocal(r1[:], en_eps[:])
        coef = persist.tile([NBD, 1], F32, tag="coef")
        nc.vector.tensor_scalar(coef[:], r1[:], -1.0, 1.0, OP.mult, OP.add)
        nrm_eps = persist.tile([NBD, 1], F32, tag="nrmeps")
        nc.vector.tensor_scalar(nrm_eps[:], nrm[:], EPS, None, OP.add)
        r2 = persist.tile([NBD, 1], F32, tag="r2")
        nc.vector.reciprocal(r2[:], nrm_eps[:])
        fac = persist.tile([NBD, 1], F32, tag="fac")
        nc.vector.tensor_tensor(fac[:], coef[:], r2[:], OP.mult)

        res_t = persist.tile([NBD, DD], F32, tag="res")
        nc.vector.tensor_scalar(res_t[:], s_diag[:], fac[:], None, OP.mult)

        nc.sync.dma_start(out_ap.rearrange("b d j -> (b d) j"), res_t[:])


_CACHE: dict = {}


def _get_nc():
    if "nc" not in _CACHE:
        nc = bacc.Bacc(
            "TRN2", target_bir_lowering=False, debug=False, num_devices=NCORES
        )
        pc = nc.dram_tensor("primary_caps", [BPC, N, DP], F32, kind="ExternalInput").ap()
        W = nc.dram_tensor("W", [D, N, DD, DP], F32, kind="ExternalInput").ap()
        Bp = nc.dram_tensor("B_prior", [D, 1, N], F32, kind="ExternalInput").ap()
        out = nc.dram_tensor("out", [BPC, D, DD], F32, kind="ExternalOutput").ap()
        with tile.TileContext(nc) as tc:
            _build_kernel(tc, out, pc, W, Bp)
        nc.compile()
        _CACHE["nc"] = nc
    return _CACHE["nc"]


def _run(primary_caps, W, B_prior, trace=False, **kw):
    nc = _get_nc()
    in_maps = [
        {
            "primary_caps": np.ascontiguousarray(
                primary_caps[c * BPC:(c + 1) * BPC]
            ).astype(np.float32),
            "W": np.asarray(W, dtype=np.float32),
            "B_prior": np.asarray(B_prior, dtype=np.float32),
        }
        for c in range(NCORES)
    ]
    res = run_bass_kernel_spmd(nc, in_maps, list(range(NCORES)), trace=trace, **kw)
    out = np.concatenate([res.results[c]["out"] for c in range(NCORES)], axis=0)
    return out.astype(np.float32), res


def kernel(primary_caps, W, B_prior):
    out, _ = _run(primary_caps, W, B_prior, trace=False)
    return out


# revision 36
# speedup vs baseline: 1.8877x; 1.0185x over previous
"""DigitCaps kernel for 8 Trainium2 NeuronCores.

Math (per batch b):
    U_hat[b,d,n,j] = sum_i W[d,n,j,i] * u[b,n,i]
    A_sum[b,d,m]   = sum_n U_hat[b,d,n,:] . U_hat[b,d,m,:] / sqrt(dp)
                   = s[b,d,:] . U_hat[b,d,m,:] / sqrt(dp),  s = sum_n U_hat
    C              = softmax_d(A_sum)
    S[b,d,j]       = sum_m (B_prior[d,m] + C[b,d,m]) * U_hat[b,d,m,j]
    out            = squash(S)

The huge [B,D,N,N] similarity matrix collapses because it is immediately
summed over n - only the n-sum s of U_hat is needed.

Sharding: data-parallel over batch, 2 batches per core, W/B_prior replicated.

Per-core layout: n-tiles of 128 on partitions.
    W_sb[nt]  : [n=128, (d,j,i)=1280]   (natural, 512B-contiguous rows)
    U2[nt]    : [n=128, (b,d,j)=320]    DVE multiply-accumulate chain over i
    s         : ones[128,128].T @ U2    (PE partition-reduce, PSUM accum; all
                                         rows of the PSUM tile equal s)
    then per n-tile (pipelined): A_sum (DVE) -> exp (ACT, scale=1/sqrt(dp))
    -> softmax_d norm (DVE) -> +B_prior -> S matmul (PE, PSUM accum)
    diagonal extract via iota mask, squash with small DVE/ACT ops.
"""

import math
import numpy as np

import concourse.bacc as bacc
import concourse.bass as bass
import concourse.tile as tile
from concourse import mybir
from concourse.bass_utils import run_bass_kernel_spmd

F32 = mybir.dt.float32
I32 = mybir.dt.int32
AX = mybir.AxisListType
OP = mybir.AluOpType
ACTF = mybir.ActivationFunctionType

B, N, DP = 16, 1152, 8
D, DD = 10, 16
NCORES = 8
BPC = B // NCORES            # 2 batches per core
NT = N // 128                # 9 n-tiles
FW = D * DD * DP             # 1280 W free size
FD = D * DD                  # 160 per-batch U2 free size
FU = BPC * FD                # 320 U2 free size
NBD = BPC * D                # 20 (b,d) pairs
EPS = 1e-7
INV_SQRT_DP = 1.0 / math.sqrt(DP)


def _build_kernel(tc: "tile.TileContext", out_ap, pc, W, Bp):
    nc = tc.nc
    with (
        tc.tile_pool(name="wpool", bufs=NT) as wpool,
        tc.tile_pool(name="upool", bufs=NT) as upool,
        tc.tile_pool(name="tapool", bufs=4) as tapool,
        tc.tile_pool(name="ppool", bufs=4) as ppool,
        tc.tile_pool(name="smpool", bufs=2) as smpool,
        tc.tile_pool(name="persist", bufs=1) as persist,
        tc.tile_pool(name="psum_s", bufs=1, space="PSUM") as psum_s,
        tc.tile_pool(name="psum_S2", bufs=1, space="PSUM") as psum_S2,
    ):
        BF16 = mybir.dt.bfloat16
        ones_t = persist.tile([128, 128], BF16, tag="ones")
        nc.vector.memset(ones_t[:], 1.0)

        u2_all = persist.tile([128, NT * FU], F32, tag="u2all")
        u2bf_all = persist.tile([128, NT * FU], BF16, tag="u2bfall")
        cbbf_all = persist.tile([128, NT * NBD], BF16, tag="cbbfall")
        bp_all = persist.tile([128, NT * D], F32, tag="bpall")
        cb_all = persist.tile([128, NT * NBD], F32, tag="cball")
        e_all = persist.tile([128, NT * NBD], F32, tag="eall")
        z_all = persist.tile([128, NT * BPC], F32, tag="zall")
        zr_all = persist.tile([128, NT * BPC], F32, tag="zrall")

        s_ps = psum_s.tile([128, FU], F32, tag="sps")

        # preload the Exp ACT table while ACT is idle (hides the ~1.3us
        # table load that would otherwise land in the phase-2 critical path)
        warm_t = persist.tile([1, 1], F32, tag="warm")
        nc.vector.memset(warm_t[:], 0.0)
        nc.scalar.activation(warm_t[:], warm_t[:], ACTF.Exp)

        # ---- phase 1: load; U2 votes via i-chain; running s on PE ----
        ACT_TILES = ()  # these tiles route products via ACT + GpSimd
        # (nt, b) half-chains routed to GpSimd (mult + tree, all Pool-legal)
        POOL_HALVES = {(1, 1), (3, 1), (5, 1), (7, 1)}
        for nt in range(NT):
            nsl = slice(nt * 128, (nt + 1) * 128)

            w_t = wpool.tile([128, FW], F32, tag="w")
            nc.sync.dma_start(w_t[:], W[:, nsl, :, :].transpose([1, 0, 2, 3]))

            u_t = upool.tile([128, BPC * DP], F32, tag="u")
            nc.sync.dma_start(u_t[:], pc[:, nsl, :].transpose([1, 0, 2]))

            # U2[n,(b,d,j)] += W[n,(d,j,i)] * u[n,(b,i)]  accumulated over i.
            # TensorScalarPtr is DVE-only on trn2 (walrus rejects it on Pool),
            # so offload tiles via ACT products + GpSimd tree-reduce instead.
            w_3 = w_t[:].rearrange("p (dj i) -> p dj i", dj=FD, i=DP)
            if nt in ACT_TILES:
                # products P[n,(b,dj,i)] on ACT (Copy with per-partition
                # scale), then i-tree-reduce on GpSimd
                pp = ppool.tile([128, BPC * FW], F32, tag="pp")
                pp_v = pp[:].rearrange(
                    "p (b dj i) -> p b dj i", b=BPC, dj=FD, i=DP
                )
                for b in range(BPC):
                    for i in range(DP):
                        nc.scalar.activation(
                            pp_v[:, b, :, i],
                            w_3[:, :, i],
                            ACTF.Copy,
                            scale=u_t[:, b * DP + i: b * DP + i + 1],
                        )
                t1 = ppool.tile([128, BPC * FD * 4], F32, tag="t1")
                t1_v = t1[:].rearrange("p (g i) -> p g i", g=BPC * FD, i=4)
                pp_g = pp[:].rearrange("p (g i) -> p g i", g=BPC * FD, i=DP)
                nc.gpsimd.tensor_tensor(
                    t1_v, pp_g[:, :, 0:4], pp_g[:, :, 4:8], OP.add
                )
                t2 = ppool.tile([128, BPC * FD * 2], F32, tag="t2")
                t2_v = t2[:].rearrange("p (g i) -> p g i", g=BPC * FD, i=2)
                nc.gpsimd.tensor_tensor(
                    t2_v, t1_v[:, :, 0:2], t1_v[:, :, 2:4], OP.add
                )
                nc.gpsimd.tensor_tensor(
                    u2_all[:, nt * FU:(nt + 1) * FU].rearrange(
                        "p (g i) -> p g i", g=BPC * FD, i=1
                    ),
                    t2_v[:, :, 0:1],
                    t2_v[:, :, 1:2],
                    OP.add,
                )
            else:
                for b in range(BPC):
                    u2_sl = u2_all[:, nt * FU + b * FD: nt * FU + (b + 1) * FD]
                    if (nt, b) in POOL_HALVES:
                        # GpSimd route: one big mult + 3 tree-adds over i
                        pp = ppool.tile([128, FW], F32, tag="pp")
                        pp_v = pp[:].rearrange("p (g i) -> p g i", g=FD, i=DP)
                        u_bc = (
                            u_t[:, b * DP:(b + 1) * DP]
                            .unsqueeze(1)
                            .broadcast_to([128, FD, DP])
                        )
                        nc.gpsimd.tensor_tensor(pp_v, w_3, u_bc, OP.mult)
                        t1 = ppool.tile([128, FD * 4], F32, tag="t1")
                        t1_v = t1[:].rearrange("p (g i) -> p g i", g=FD, i=4)
                        nc.gpsimd.tensor_tensor(
                            t1_v, pp_v[:, :, 0:4], pp_v[:, :, 4:8], OP.add
                        )
                        t2 = ppool.tile([128, FD * 2], F32, tag="t2")
                        t2_v = t2[:].rearrange("p (g i) -> p g i", g=FD, i=2)
                        nc.gpsimd.tensor_tensor(
                            t2_v, t1_v[:, :, 0:2], t1_v[:, :, 2:4], OP.add
                        )
                        nc.gpsimd.tensor_tensor(
                            u2_sl.rearrange("p (g i) -> p g i", g=FD, i=1),
                            t2_v[:, :, 0:1],
                            t2_v[:, :, 1:2],
                            OP.add,
                        )
                        continue
                    # first product on ACT (Copy with per-partition scale)
                    # frees two DVE ops per tile
                    nc.scalar.activation(
                        u2_sl,
                        w_3[:, :, 0],
                        ACTF.Copy,
                        scale=u_t[:, b * DP: b * DP + 1],
                    )
                    for i in range(1, DP):
                        nc.vector.scalar_tensor_tensor(
                            u2_sl,
                            w_3[:, :, i],
                            u_t[:, b * DP + i: b * DP + i + 1],
                            u2_sl,
                            OP.mult,
                            OP.add,
                        )

            # bf16 shadow copy of U2 for the PE matmuls (ACT is idle)
            u2bf_sl = u2bf_all[:, nt * FU:(nt + 1) * FU]
            nc.scalar.copy(u2bf_sl, u2_all[:, nt * FU:(nt + 1) * FU])

            # s accumulation: every row of s_ps ends up = sum_n U2[n, :]
            nc.tensor.matmul(
                s_ps[:],
                ones_t[:],
                u2bf_sl,
                start=(nt == 0),
                stop=(nt == NT - 1),
            )

        # B_prior loads only matter in phase 2 - keep them off the W/u stream
        for nt in range(NT):
            nc.sync.dma_start(
                bp_all[:, nt * D:(nt + 1) * D],
                Bp[:, 0, nt * 128:(nt + 1) * 128].transpose([1, 0]),
            )

        # ---- phase 2 (pipelined per n-tile): A_sum -> softmax_d -> +B_prior
        #      -> S matmul ----
        # s copy to SBUF so GpSimd (no PSUM access) can read it
        s_sb = persist.tile([128, FU], F32, tag="ssb")
        nc.scalar.copy(s_sb[:], s_ps[:])

        S2_ps = psum_S2.tile([NBD, FU], F32, tag="S2")
        POOL_TILES = (3, 4, 5, 6, 7, 8)  # TA on GpSimd for these n-tiles
        for nt in range(NT):
            u2_sl = u2_all[:, nt * FU:(nt + 1) * FU]
            a_sl = e_all[:, nt * NBD:(nt + 1) * NBD]  # staging (overwritten by exp)
            ta = tapool.tile([128, FU], F32, tag="ta")
            if nt in POOL_TILES:
                nc.gpsimd.tensor_tensor(ta[:], u2_sl, s_sb[:], OP.mult)
            else:
                nc.vector.tensor_tensor(ta[:], u2_sl, s_ps[:], OP.mult)
            nc.vector.tensor_reduce(
                a_sl,
                ta[:].rearrange("p (g j) -> p g j", g=NBD, j=DD),
                AX.X,
                OP.add,
            )
            # E = exp(A / sqrt(dp))
            nc.scalar.activation(a_sl, a_sl, ACTF.Exp, scale=INV_SQRT_DP)
            # z[(b)] = sum_d E ; zr = 1/z
            z_sl = z_all[:, nt * BPC:(nt + 1) * BPC]
            zr_sl = zr_all[:, nt * BPC:(nt + 1) * BPC]
            nc.vector.tensor_reduce(
                z_sl,
                a_sl.rearrange("p (b d) -> p b d", b=BPC, d=D),
                AX.X,
                OP.add,
            )
            nc.vector.reciprocal(zr_sl, z_sl)
            # cb = E * zr + B_prior, written directly as bf16 for the matmul
            cbbf_sl = cbbf_all[:, nt * NBD:(nt + 1) * NBD]
            for b in range(BPC):
                nc.vector.scalar_tensor_tensor(
                    cbbf_sl[:, b * D:(b + 1) * D],
                    a_sl[:, b * D:(b + 1) * D],
                    zr_sl[:, b: b + 1],
                    bp_all[:, nt * D:(nt + 1) * D],
                    OP.mult,
                    OP.add,
                )
            # S2 += cb.T @ U2 (bf16 operands, fp32 PSUM accumulate)
            nc.tensor.matmul(
                S2_ps[:],
                cbbf_sl,
                u2bf_all[:, nt * FU:(nt + 1) * FU],
                start=(nt == 0),
                stop=(nt == NT - 1),
            )

        # ---- phase 3: extract diagonal (b,d)=(b',d') via iota mask ----
        iota_t = persist.tile([NBD, FU], I32, tag="iota")
        nc.gpsimd.iota(
            iota_t[:], pattern=[[1, NBD], [0, DD]], base=0, channel_multiplier=-1
        )
        mask_t = persist.tile([NBD, FU], F32, tag="mask")
        nc.vector.tensor_scalar(mask_t[:], iota_t[:], 0, None, OP.is_equal)

        sm_t = smpool.tile([NBD, FU], F32, tag="sm")
        nc.vector.tensor_tensor(sm_t[:], S2_ps[:], mask_t[:], OP.mult)
        s_diag = persist.tile([NBD, DD], F32, tag="sdiag")
        nc.vector.tensor_reduce(
            s_diag[:],
            sm_t[:].rearrange("p (g j) -> p j g", g=NBD, j=DD),
            AX.X,
            OP.add,
        )

        # ---- phase 4: squash ----
        ss_t = persist.tile([NBD, DD], F32, tag="ss")
        nrm2 = persist.tile([NBD, 1], F32, tag="nrm2")
        nc.vector.tensor_tensor(ss_t[:], s_diag[:], s_diag[:], OP.mult)
        nc.vector.tensor_reduce(nrm2[:], ss_t[:], AX.X, OP.add)
        # norm via DVE Newton sqrt (bit-hack seed + 2 iterations) - keeps the
        # Exp ACT table resident (no sqrt/exp table reload in the tail)
        nrm = persist.tile([NBD, 1], F32, tag="nrm")
        seed_i = persist.tile([NBD, 1], I32, tag="seedi")
        nc.vector.tensor_scalar(
            seed_i[:], nrm2[:].bitcast(I32), 1, None, OP.logical_shift_right
        )
        nc.vector.tensor_scalar(seed_i[:], seed_i[:], 0x1FBD1DF5, None, OP.add)
        nc.vector.tensor_copy(nrm[:], seed_i[:].bitcast(F32))
        nwr = persist.tile([NBD, 1], F32, tag="nwr")
        nwt = persist.tile([NBD, 1], F32, tag="nwt")
        for _ in range(2):
            nc.vector.reciprocal(nwr[:], nrm[:])
            nc.vector.tensor_tensor(nwt[:], nrm2[:], nwr[:], OP.mult)
            nc.vector.tensor_tensor(nrm[:], nrm[:], nwt[:], OP.add)
            nc.vector.tensor_scalar(nrm[:], nrm[:], 0.5, None, OP.mult)
        en = persist.tile([NBD, 1], F32, tag="en")
        nc.scalar.activation(en[:], nrm[:], ACTF.Exp)
        en_eps = persist.tile([NBD, 1], F32, tag="eneps")
        nc.vector.tensor_scalar(en_eps[:], en[:], EPS, None, OP.add)
        r1 = persist.tile([NBD, 1], F32, tag="r1")
        nc.vector.reciprocal(r1[:], en_eps[:])
        coef = persist.tile([NBD, 1], F32, tag="coef")
        nc.vector.tensor_scalar(coef[:], r1[:], -1.0, 1.0, OP.mult, OP.add)
        nrm_eps = persist.tile([NBD, 1], F32, tag="nrmeps")
        nc.vector.tensor_scalar(nrm_eps[:], nrm[:], EPS, None, OP.add)
        r2 = persist.tile([NBD, 1], F32, tag="r2")
        nc.vector.reciprocal(r2[:], nrm_eps[:])
        fac = persist.tile([NBD, 1], F32, tag="fac")
        nc.vector.tensor_tensor(fac[:], coef[:], r2[:], OP.mult)

        res_t = persist.tile([NBD, DD], F32, tag="res")
        nc.vector.tensor_scalar(res_t[:], s_diag[:], fac[:], None, OP.mult)

        nc.sync.dma_start(out_ap.rearrange("b d j -> (b d) j"), res_t[:])


_CACHE: dict = {}


def _get_nc():
    if "nc" not in _CACHE:
        nc = bacc.Bacc(
            "TRN2", target_bir_lowering=False, debug=False, num_devices=NCORES
        )
        pc = nc.dram_tensor("primary_caps", [BPC, N, DP], F32, kind="ExternalInput").ap()
        W = nc.dram_tensor("W", [D, N, DD, DP], F32, kind="ExternalInput").ap()
        Bp = nc.dram_tensor("B_prior", [D, 1, N], F32, kind="ExternalInput").ap()
        out = nc.dram_tensor("out", [BPC, D, DD], F32, kind="ExternalOutput").ap()
        with tile.TileContext(nc) as tc:
            _build_kernel(tc, out, pc, W, Bp)
        nc.compile()
        _CACHE["nc"] = nc
    return _CACHE["nc"]


def _run(primary_caps, W, B_prior, trace=False, **kw):
    nc = _get_nc()
    in_maps = [
        {
            "primary_caps": np.ascontiguousarray(
                primary_caps[c * BPC:(c + 1) * BPC]
            ).astype(np.float32),
            "W": np.asarray(W, dtype=np.float32),
            "B_prior": np.asarray(B_prior, dtype=np.float32),
        }
        for c in range(NCORES)
    ]
    res = run_bass_kernel_spmd(nc, in_maps, list(range(NCORES)), trace=trace, **kw)
    out = np.concatenate([res.results[c]["out"] for c in range(NCORES)], axis=0)
    return out.astype(np.float32), res


def kernel(primary_caps, W, B_prior):
    out, _ = _run(primary_caps, W, B_prior, trace=False)
    return out


# revision 47
# speedup vs baseline: 1.9097x; 1.0117x over previous
"""DigitCaps kernel for 8 Trainium2 NeuronCores.

Math (per batch b):
    U_hat[b,d,n,j] = sum_i W[d,n,j,i] * u[b,n,i]
    A_sum[b,d,m]   = sum_n U_hat[b,d,n,:] . U_hat[b,d,m,:] / sqrt(dp)
                   = s[b,d,:] . U_hat[b,d,m,:] / sqrt(dp),  s = sum_n U_hat
    C              = softmax_d(A_sum)
    S[b,d,j]       = sum_m (B_prior[d,m] + C[b,d,m]) * U_hat[b,d,m,j]
    out            = squash(S)

The huge [B,D,N,N] similarity matrix collapses because it is immediately
summed over n - only the n-sum s of U_hat is needed.

Sharding: data-parallel over batch, 2 batches per core, W/B_prior replicated.

Per-core layout: n-tiles of 128 on partitions.
    W_sb[nt]  : [n=128, (d,j,i)=1280]   (natural, 512B-contiguous rows)
    U2[nt]    : [n=128, (b,d,j)=320]    DVE multiply-accumulate chain over i
    s         : ones[128,128].T @ U2    (PE partition-reduce, PSUM accum; all
                                         rows of the PSUM tile equal s)
    then per n-tile (pipelined): A_sum (DVE) -> exp (ACT, scale=1/sqrt(dp))
    -> softmax_d norm (DVE) -> +B_prior -> S matmul (PE, PSUM accum)
    diagonal extract via iota mask, squash with small DVE/ACT ops.
"""

import math
import numpy as np

import concourse.bacc as bacc
import concourse.bass as bass
import concourse.tile as tile
from concourse import mybir
from concourse.bass_utils import run_bass_kernel_spmd

F32 = mybir.dt.float32
I32 = mybir.dt.int32
AX = mybir.AxisListType
OP = mybir.AluOpType
ACTF = mybir.ActivationFunctionType

B, N, DP = 16, 1152, 8
D, DD = 10, 16
NCORES = 8
BPC = B // NCORES            # 2 batches per core
NT = N // 128                # 9 n-tiles
FW = D * DD * DP             # 1280 W free size
FD = D * DD                  # 160 per-batch U2 free size
FU = BPC * FD                # 320 U2 free size
NBD = BPC * D                # 20 (b,d) pairs
EPS = 1e-7
INV_SQRT_DP = 1.0 / math.sqrt(DP)


def _build_kernel(tc: "tile.TileContext", out_ap, pc, W, Bp):
    nc = tc.nc
    with (
        tc.tile_pool(name="wpool", bufs=NT) as wpool,
        tc.tile_pool(name="upool", bufs=NT) as upool,
        tc.tile_pool(name="tapool", bufs=6) as tapool,
        tc.tile_pool(name="ppool", bufs=4) as ppool,
        tc.tile_pool(name="smpool", bufs=2) as smpool,
        tc.tile_pool(name="persist", bufs=1) as persist,
        tc.tile_pool(name="psum_s", bufs=1, space="PSUM") as psum_s,
        tc.tile_pool(name="psum_S2", bufs=1, space="PSUM") as psum_S2,
    ):
        BF16 = mybir.dt.bfloat16
        ones_t = persist.tile([128, 128], BF16, tag="ones")
        nc.vector.memset(ones_t[:], 1.0)

        u2_all = persist.tile([128, NT * FU], F32, tag="u2all")
        u2bf_all = persist.tile([128, NT * FU], BF16, tag="u2bfall")
        cbbf_all = persist.tile([128, NT * NBD], BF16, tag="cbbfall")
        bp_all = persist.tile([128, NT * D], F32, tag="bpall")
        cb_all = persist.tile([128, NT * NBD], F32, tag="cball")
        e_all = persist.tile([128, NT * NBD], F32, tag="eall")
        z_all = persist.tile([128, NT * BPC], F32, tag="zall")
        zr_all = persist.tile([128, NT * BPC], F32, tag="zrall")

        s_ps = psum_s.tile([128, FU], F32, tag="sps")

        # preload the Exp ACT table while ACT is idle (hides the ~1.3us
        # table load that would otherwise land in the phase-2 critical path)
        warm_t = persist.tile([1, 1], F32, tag="warm")
        nc.vector.memset(warm_t[:], 0.0)
        nc.scalar.activation(warm_t[:], warm_t[:], ACTF.Exp)

        # ---- phase 1: load; U2 votes via i-chain; running s on PE ----
        ACT_TILES = ()  # these tiles route products via ACT + GpSimd
        # (nt, b) half-chains routed to GpSimd (mult + tree, all Pool-legal)
        POOL_HALVES = {(1, 1), (3, 1), (5, 1), (7, 1)}
        # halves computed as ACT products + one DVE segmented reduce
        ACT_HALVES = set()
        for nt in range(NT):
            nsl = slice(nt * 128, (nt + 1) * 128)

            w_t = wpool.tile([128, FW], F32, tag="w")
            nc.sync.dma_start(w_t[:], W[:, nsl, :, :].transpose([1, 0, 2, 3]))

            u_t = upool.tile([128, BPC * DP], F32, tag="u")
            nc.sync.dma_start(u_t[:], pc[:, nsl, :].transpose([1, 0, 2]))

            # U2[n,(b,d,j)] += W[n,(d,j,i)] * u[n,(b,i)]  accumulated over i.
            # TensorScalarPtr is DVE-only on trn2 (walrus rejects it on Pool),
            # so offload tiles via ACT products + GpSimd tree-reduce instead.
            w_3 = w_t[:].rearrange("p (dj i) -> p dj i", dj=FD, i=DP)
            if nt in ACT_TILES:
                # products P[n,(b,dj,i)] on ACT (Copy with per-partition
                # scale), then i-tree-reduce on GpSimd
                pp = ppool.tile([128, BPC * FW], F32, tag="pp")
                pp_v = pp[:].rearrange(
                    "p (b dj i) -> p b dj i", b=BPC, dj=FD, i=DP
                )
                for b in range(BPC):
                    for i in range(DP):
                        nc.scalar.activation(
                            pp_v[:, b, :, i],
                            w_3[:, :, i],
                            ACTF.Copy,
                            scale=u_t[:, b * DP + i: b * DP + i + 1],
                        )
                t1 = ppool.tile([128, BPC * FD * 4], F32, tag="t1")
                t1_v = t1[:].rearrange("p (g i) -> p g i", g=BPC * FD, i=4)
                pp_g = pp[:].rearrange("p (g i) -> p g i", g=BPC * FD, i=DP)
                nc.gpsimd.tensor_tensor(
                    t1_v, pp_g[:, :, 0:4], pp_g[:, :, 4:8], OP.add
                )
                t2 = ppool.tile([128, BPC * FD * 2], F32, tag="t2")
                t2_v = t2[:].rearrange("p (g i) -> p g i", g=BPC * FD, i=2)
                nc.gpsimd.tensor_tensor(
                    t2_v, t1_v[:, :, 0:2], t1_v[:, :, 2:4], OP.add
                )
                nc.gpsimd.tensor_tensor(
                    u2_all[:, nt * FU:(nt + 1) * FU].rearrange(
                        "p (g i) -> p g i", g=BPC * FD, i=1
                    ),
                    t2_v[:, :, 0:1],
                    t2_v[:, :, 1:2],
                    OP.add,
                )
            else:
                for b in range(BPC):
                    u2_sl = u2_all[:, nt * FU + b * FD: nt * FU + (b + 1) * FD]
                    if (nt, b) in ACT_HALVES:
                        # 8 ACT products (Copy w/ per-partition scale) into a
                        # products tile, then one DVE segmented reduce over i
                        pa = ppool.tile([128, FW], F32, tag="pa")
                        pa_v = pa[:].rearrange("p (g i) -> p g i", g=FD, i=DP)
                        for i in range(DP):
                            nc.scalar.activation(
                                pa_v[:, :, i],
                                w_3[:, :, i],
                                ACTF.Copy,
                                scale=u_t[:, b * DP + i: b * DP + i + 1],
                            )
                        nc.vector.tensor_reduce(u2_sl, pa_v, AX.X, OP.add)
                        continue
                    if (nt, b) in POOL_HALVES:
                        # GpSimd route: one big mult + 3 tree-adds over i
                        pp = ppool.tile([128, FW], F32, tag="pp")
                        pp_v = pp[:].rearrange("p (g i) -> p g i", g=FD, i=DP)
                        u_bc = (
                            u_t[:, b * DP:(b + 1) * DP]
                            .unsqueeze(1)
                            .broadcast_to([128, FD, DP])
                        )
                        nc.gpsimd.tensor_tensor(pp_v, w_3, u_bc, OP.mult)
                        t1 = ppool.tile([128, FD * 4], F32, tag="t1")
                        t1_v = t1[:].rearrange("p (g i) -> p g i", g=FD, i=4)
                        nc.gpsimd.tensor_tensor(
                            t1_v, pp_v[:, :, 0:4], pp_v[:, :, 4:8], OP.add
                        )
                        t2 = ppool.tile([128, FD * 2], F32, tag="t2")
                        t2_v = t2[:].rearrange("p (g i) -> p g i", g=FD, i=2)
                        nc.gpsimd.tensor_tensor(
                            t2_v, t1_v[:, :, 0:2], t1_v[:, :, 2:4], OP.add
                        )
                        nc.gpsimd.tensor_tensor(
                            u2_sl.rearrange("p (g i) -> p g i", g=FD, i=1),
                            t2_v[:, :, 0:1],
                            t2_v[:, :, 1:2],
                            OP.add,
                        )
                        continue
                    # first product on ACT (Copy with per-partition scale)
                    # frees two DVE ops per tile
                    nc.scalar.activation(
                        u2_sl,
                        w_3[:, :, 0],
                        ACTF.Copy,
                        scale=u_t[:, b * DP: b * DP + 1],
                    )
                    for i in range(1, DP):
                        nc.vector.scalar_tensor_tensor(
                            u2_sl,
                            w_3[:, :, i],
                            u_t[:, b * DP + i: b * DP + i + 1],
                            u2_sl,
                            OP.mult,
                            OP.add,
                        )

            # bf16 shadow copy of U2 for the PE matmuls (ACT is idle)
            u2bf_sl = u2bf_all[:, nt * FU:(nt + 1) * FU]
            nc.scalar.copy(u2bf_sl, u2_all[:, nt * FU:(nt + 1) * FU])

            # s accumulation: every row of s_ps ends up = sum_n U2[n, :]
            nc.tensor.matmul(
                s_ps[:],
                ones_t[:],
                u2bf_sl,
                start=(nt == 0),
                stop=(nt == NT - 1),
            )

        # B_prior loads only matter in phase 2 - keep them off the W/u stream
        for nt in range(NT):
            nc.sync.dma_start(
                bp_all[:, nt * D:(nt + 1) * D],
                Bp[:, 0, nt * 128:(nt + 1) * 128].transpose([1, 0]),
            )

        # ---- phase 2 (pipelined per n-tile): A_sum -> softmax_d -> +B_prior
        #      -> S matmul ----
        # s copy to SBUF so GpSimd (no PSUM access) can read it
        s_sb = persist.tile([128, FU], F32, tag="ssb")
        nc.scalar.copy(s_sb[:], s_ps[:])

        S2_ps = psum_S2.tile([NBD, FU], F32, tag="S2")
        POOL_TILES = (1, 2, 3, 4, 5, 6, 7, 8)  # TA on GpSimd for these n-tiles
        for nt in range(NT):
            u2_sl = u2_all[:, nt * FU:(nt + 1) * FU]
            a_sl = e_all[:, nt * NBD:(nt + 1) * NBD]  # staging (overwritten by exp)
            ta = tapool.tile([128, FU], F32, tag="ta")
            if nt in POOL_TILES:
                nc.gpsimd.tensor_tensor(ta[:], u2_sl, s_sb[:], OP.mult)
            else:
                nc.vector.tensor_tensor(ta[:], u2_sl, s_ps[:], OP.mult)
            nc.vector.tensor_reduce(
                a_sl,
                ta[:].rearrange("p (g j) -> p g j", g=NBD, j=DD),
                AX.X,
                OP.add,
            )
            # E = exp(A / sqrt(dp))
            nc.scalar.activation(a_sl, a_sl, ACTF.Exp, scale=INV_SQRT_DP)
            # z[(b)] = sum_d E ; zr = 1/z
            z_sl = z_all[:, nt * BPC:(nt + 1) * BPC]
            zr_sl = zr_all[:, nt * BPC:(nt + 1) * BPC]
            nc.vector.tensor_reduce(
                z_sl,
                a_sl.rearrange("p (b d) -> p b d", b=BPC, d=D),
                AX.X,
                OP.add,
            )
            nc.vector.reciprocal(zr_sl, z_sl)
            # cb = E * zr + B_prior, written directly as bf16 for the matmul
            cbbf_sl = cbbf_all[:, nt * NBD:(nt + 1) * NBD]
            for b in range(BPC):
                nc.vector.scalar_tensor_tensor(
                    cbbf_sl[:, b * D:(b + 1) * D],
                    a_sl[:, b * D:(b + 1) * D],
                    zr_sl[:, b: b + 1],
                    bp_all[:, nt * D:(nt + 1) * D],
                    OP.mult,
                    OP.add,
                )
            # S2 += cb.T @ U2 (bf16 operands, fp32 PSUM accumulate)
            nc.tensor.matmul(
                S2_ps[:],
                cbbf_sl,
                u2bf_all[:, nt * FU:(nt + 1) * FU],
                start=(nt == 0),
                stop=(nt == NT - 1),
            )

        # ---- phase 3: extract diagonal (b,d)=(b',d') via iota mask ----
        iota_t = persist.tile([NBD, FU], I32, tag="iota")
        nc.gpsimd.iota(
            iota_t[:], pattern=[[1, NBD], [0, DD]], base=0, channel_multiplier=-1
        )
        mask_t = persist.tile([NBD, FU], F32, tag="mask")
        nc.vector.tensor_scalar(mask_t[:], iota_t[:], 0, None, OP.is_equal)

        sm_t = smpool.tile([NBD, FU], F32, tag="sm")
        nc.vector.tensor_tensor(sm_t[:], S2_ps[:], mask_t[:], OP.mult)
        s_diag = persist.tile([NBD, DD], F32, tag="sdiag")
        nc.vector.tensor_reduce(
            s_diag[:],
            sm_t[:].rearrange("p (g j) -> p j g", g=NBD, j=DD),
            AX.X,
            OP.add,
        )

        # ---- phase 4: squash ----
        ss_t = persist.tile([NBD, DD], F32, tag="ss")
        nrm2 = persist.tile([NBD, 1], F32, tag="nrm2")
        nc.vector.tensor_tensor(ss_t[:], s_diag[:], s_diag[:], OP.mult)
        nc.vector.tensor_reduce(nrm2[:], ss_t[:], AX.X, OP.add)
        # norm via DVE Newton sqrt (bit-hack seed + 2 iterations) - keeps the
        # Exp ACT table resident (no sqrt/exp table reload in the tail)
        nrm = persist.tile([NBD, 1], F32, tag="nrm")
        seed_i = persist.tile([NBD, 1], I32, tag="seedi")
        nc.vector.tensor_scalar(
            seed_i[:], nrm2[:].bitcast(I32), 1, None, OP.logical_shift_right
        )
        nc.vector.tensor_scalar(seed_i[:], seed_i[:], 0x1FBD1DF5, None, OP.add)
        nc.vector.tensor_copy(nrm[:], seed_i[:].bitcast(F32))
        nwr = persist.tile([NBD, 1], F32, tag="nwr")
        nwt = persist.tile([NBD, 1], F32, tag="nwt")
        for _ in range(2):
            nc.vector.reciprocal(nwr[:], nrm[:])
            nc.vector.tensor_tensor(nwt[:], nrm2[:], nwr[:], OP.mult)
            nc.vector.tensor_tensor(nrm[:], nrm[:], nwt[:], OP.add)
            nc.vector.tensor_scalar(nrm[:], nrm[:], 0.5, None, OP.mult)
        en = persist.tile([NBD, 1], F32, tag="en")
        nc.scalar.activation(en[:], nrm[:], ACTF.Exp)
        en_eps = persist.tile([NBD, 1], F32, tag="eneps")
        nc.vector.tensor_scalar(en_eps[:], en[:], EPS, None, OP.add)
        r1 = persist.tile([NBD, 1], F32, tag="r1")
        nc.vector.reciprocal(r1[:], en_eps[:])
        coef = persist.tile([NBD, 1], F32, tag="coef")
        nc.vector.tensor_scalar(coef[:], r1[:], -1.0, 1.0, OP.mult, OP.add)
        nrm_eps = persist.tile([NBD, 1], F32, tag="nrmeps")
        nc.vector.tensor_scalar(nrm_eps[:], nrm[:], EPS, None, OP.add)
        r2 = persist.tile([NBD, 1], F32, tag="r2")
        nc.vector.reciprocal(r2[:], nrm_eps[:])
        fac = persist.tile([NBD, 1], F32, tag="fac")
        nc.vector.tensor_tensor(fac[:], coef[:], r2[:], OP.mult)

        res_t = persist.tile([NBD, DD], F32, tag="res")
        nc.vector.tensor_scalar(res_t[:], s_diag[:], fac[:], None, OP.mult)

        nc.sync.dma_start(out_ap.rearrange("b d j -> (b d) j"), res_t[:])


_CACHE: dict = {}


def _get_nc():
    if "nc" not in _CACHE:
        nc = bacc.Bacc(
            "TRN2", target_bir_lowering=False, debug=False, num_devices=NCORES
        )
        pc = nc.dram_tensor("primary_caps", [BPC, N, DP], F32, kind="ExternalInput").ap()
        W = nc.dram_tensor("W", [D, N, DD, DP], F32, kind="ExternalInput").ap()
        Bp = nc.dram_tensor("B_prior", [D, 1, N], F32, kind="ExternalInput").ap()
        out = nc.dram_tensor("out", [BPC, D, DD], F32, kind="ExternalOutput").ap()
        with tile.TileContext(nc) as tc:
            _build_kernel(tc, out, pc, W, Bp)
        nc.compile()
        _CACHE["nc"] = nc
    return _CACHE["nc"]


def _run(primary_caps, W, B_prior, trace=False, **kw):
    nc = _get_nc()
    in_maps = [
        {
            "primary_caps": np.ascontiguousarray(
                primary_caps[c * BPC:(c + 1) * BPC]
            ).astype(np.float32),
            "W": np.asarray(W, dtype=np.float32),
            "B_prior": np.asarray(B_prior, dtype=np.float32),
        }
        for c in range(NCORES)
    ]
    res = run_bass_kernel_spmd(nc, in_maps, list(range(NCORES)), trace=trace, **kw)
    out = np.concatenate([res.results[c]["out"] for c in range(NCORES)], axis=0)
    return out.astype(np.float32), res


def kernel(primary_caps, W, B_prior):
    out, _ = _run(primary_caps, W, B_prior, trace=False)
    return out


# revision 55
# speedup vs baseline: 1.9358x; 1.0137x over previous
"""DigitCaps kernel for 8 Trainium2 NeuronCores.

Math (per batch b):
    U_hat[b,d,n,j] = sum_i W[d,n,j,i] * u[b,n,i]
    A_sum[b,d,m]   = sum_n U_hat[b,d,n,:] . U_hat[b,d,m,:] / sqrt(dp)
                   = s[b,d,:] . U_hat[b,d,m,:] / sqrt(dp),  s = sum_n U_hat
    C              = softmax_d(A_sum)
    S[b,d,j]       = sum_m (B_prior[d,m] + C[b,d,m]) * U_hat[b,d,m,j]
    out            = squash(S)

The huge [B,D,N,N] similarity matrix collapses because it is immediately
summed over n - only the n-sum s of U_hat is needed.

Sharding: data-parallel over batch, 2 batches per core, W/B_prior replicated.

Per-core layout: n-tiles of 128 on partitions.
    W_sb[nt]  : [n=128, (d,j,i)=1280]   (natural, 512B-contiguous rows)
    U2[nt]    : [n=128, (b,d,j)=320]    DVE multiply-accumulate chain over i
    s         : ones[128,128].T @ U2    (PE partition-reduce, PSUM accum; all
                                         rows of the PSUM tile equal s)
    then per n-tile (pipelined): A_sum (DVE) -> exp (ACT, scale=1/sqrt(dp))
    -> softmax_d norm (DVE) -> +B_prior -> S matmul (PE, PSUM accum)
    diagonal extract via iota mask, squash with small DVE/ACT ops.
"""

import math
import numpy as np

import concourse.bacc as bacc
import concourse.bass as bass
import concourse.tile as tile
from concourse import mybir
from concourse.bass_utils import run_bass_kernel_spmd

F32 = mybir.dt.float32
I32 = mybir.dt.int32
AX = mybir.AxisListType
OP = mybir.AluOpType
ACTF = mybir.ActivationFunctionType

B, N, DP = 16, 1152, 8
D, DD = 10, 16
NCORES = 8
BPC = B // NCORES            # 2 batches per core
NT = N // 128                # 9 n-tiles
FW = D * DD * DP             # 1280 W free size
FD = D * DD                  # 160 per-batch U2 free size
FU = BPC * FD                # 320 U2 free size
NBD = BPC * D                # 20 (b,d) pairs
EPS = 1e-7
INV_SQRT_DP = 1.0 / math.sqrt(DP)


def _build_kernel(tc: "tile.TileContext", out_ap, pc, W, Bp):
    nc = tc.nc
    with (
        tc.tile_pool(name="wpool", bufs=NT) as wpool,
        tc.tile_pool(name="upool", bufs=NT) as upool,
        tc.tile_pool(name="tapool", bufs=6) as tapool,
        tc.tile_pool(name="ppool", bufs=4) as ppool,
        tc.tile_pool(name="smpool", bufs=2) as smpool,
        tc.tile_pool(name="persist", bufs=1) as persist,
        tc.tile_pool(name="psum_s", bufs=1, space="PSUM") as psum_s,
        tc.tile_pool(name="psum_S2", bufs=1, space="PSUM") as psum_S2,
    ):
        BF16 = mybir.dt.bfloat16
        ones_t = persist.tile([128, 128], F32, tag="ones")
        nc.vector.memset(ones_t[:], 1.0)

        u2_all = persist.tile([128, NT * FU], F32, tag="u2all")
        u2bf_all = persist.tile([128, NT * FU], BF16, tag="u2bfall")
        cbbf_all = persist.tile([128, NT * NBD], BF16, tag="cbbfall")
        bp_all = persist.tile([128, NT * D], F32, tag="bpall")
        cb_all = persist.tile([128, NT * NBD], F32, tag="cball")
        e_all = persist.tile([128, NT * NBD], F32, tag="eall")
        z_all = persist.tile([128, NT * BPC], F32, tag="zall")
        zr_all = persist.tile([128, NT * BPC], F32, tag="zrall")

        s_ps_0 = psum_s.tile([128, FD], F32, tag="sps0")
        s_ps_1 = psum_s.tile([128, FD], F32, tag="sps1")
        s_ps_b = [s_ps_0, s_ps_1]

        # preload the Exp ACT table while ACT is idle (hides the ~1.3us
        # table load that would otherwise land in the phase-2 critical path)
        warm_t = persist.tile([1, 1], F32, tag="warm")
        nc.vector.memset(warm_t[:], 0.0)
        nc.scalar.activation(warm_t[:], warm_t[:], ACTF.Exp)

        # ---- phase 1: load; U2 votes via i-chain; running s on PE ----
        ACT_TILES = ()  # these tiles route products via ACT + GpSimd
        # (nt, b) half-chains routed to GpSimd (mult + tree, all Pool-legal)
        POOL_HALVES = {(1, 1), (3, 1), (5, 1), (7, 1)}
        for nt in range(NT):
            nsl = slice(nt * 128, (nt + 1) * 128)

            w_t = wpool.tile([128, FW], F32, tag="w")
            nc.sync.dma_start(w_t[:], W[:, nsl, :, :].transpose([1, 0, 2, 3]))

            u_t = upool.tile([128, BPC * DP], F32, tag="u")
            nc.sync.dma_start(u_t[:], pc[:, nsl, :].transpose([1, 0, 2]))

            # U2[n,(b,d,j)] += W[n,(d,j,i)] * u[n,(b,i)]  accumulated over i.
            # TensorScalarPtr is DVE-only on trn2 (walrus rejects it on Pool),
            # so offload tiles via ACT products + GpSimd tree-reduce instead.
            w_3 = w_t[:].rearrange("p (dj i) -> p dj i", dj=FD, i=DP)
            if nt in ACT_TILES:
                # products P[n,(b,dj,i)] on ACT (Copy with per-partition
                # scale), then i-tree-reduce on GpSimd
                pp = ppool.tile([128, BPC * FW], F32, tag="pp")
                pp_v = pp[:].rearrange(
                    "p (b dj i) -> p b dj i", b=BPC, dj=FD, i=DP
                )
                for b in range(BPC):
                    for i in range(DP):
                        nc.scalar.activation(
                            pp_v[:, b, :, i],
                            w_3[:, :, i],
                            ACTF.Copy,
                            scale=u_t[:, b * DP + i: b * DP + i + 1],
                        )
                t1 = ppool.tile([128, BPC * FD * 4], F32, tag="t1")
                t1_v = t1[:].rearrange("p (g i) -> p g i", g=BPC * FD, i=4)
                pp_g = pp[:].rearrange("p (g i) -> p g i", g=BPC * FD, i=DP)
                nc.gpsimd.tensor_tensor(
                    t1_v, pp_g[:, :, 0:4], pp_g[:, :, 4:8], OP.add
                )
                t2 = ppool.tile([128, BPC * FD * 2], F32, tag="t2")
                t2_v = t2[:].rearrange("p (g i) -> p g i", g=BPC * FD, i=2)
                nc.gpsimd.tensor_tensor(
                    t2_v, t1_v[:, :, 0:2], t1_v[:, :, 2:4], OP.add
                )
                nc.gpsimd.tensor_tensor(
                    u2_all[:, nt * FU:(nt + 1) * FU].rearrange(
                        "p (g i) -> p g i", g=BPC * FD, i=1
                    ),
                    t2_v[:, :, 0:1],
                    t2_v[:, :, 1:2],
                    OP.add,
                )
            else:
                for b in range(BPC):
                    u2_sl = u2_all[:, nt * FU + b * FD: nt * FU + (b + 1) * FD]
                    if (nt, b) in POOL_HALVES:
                        # GpSimd route: one big mult + 3 tree-adds over i
                        pp = ppool.tile([128, FW], F32, tag="pp")
                        pp_v = pp[:].rearrange("p (g i) -> p g i", g=FD, i=DP)
                        u_bc = (
                            u_t[:, b * DP:(b + 1) * DP]
                            .unsqueeze(1)
                            .broadcast_to([128, FD, DP])
                        )
                        nc.gpsimd.tensor_tensor(pp_v, w_3, u_bc, OP.mult)
                        t1 = ppool.tile([128, FD * 4], F32, tag="t1")
                        t1_v = t1[:].rearrange("p (g i) -> p g i", g=FD, i=4)
                        nc.gpsimd.tensor_tensor(
                            t1_v, pp_v[:, :, 0:4], pp_v[:, :, 4:8], OP.add
                        )
                        t2 = ppool.tile([128, FD * 2], F32, tag="t2")
                        t2_v = t2[:].rearrange("p (g i) -> p g i", g=FD, i=2)
                        nc.gpsimd.tensor_tensor(
                            t2_v, t1_v[:, :, 0:2], t1_v[:, :, 2:4], OP.add
                        )
                        nc.gpsimd.tensor_tensor(
                            u2_sl.rearrange("p (g i) -> p g i", g=FD, i=1),
                            t2_v[:, :, 0:1],
                            t2_v[:, :, 1:2],
                            OP.add,
                        )
                        nc.tensor.matmul(
                            s_ps_b[b][:],
                            ones_t[:],
                            u2_sl,
                            start=(nt == 0),
                            stop=(nt == NT - 1),
                        )
                        continue
                    # first product on ACT (Copy with per-partition scale)
                    # frees two DVE ops per tile
                    nc.scalar.activation(
                        u2_sl,
                        w_3[:, :, 0],
                        ACTF.Copy,
                        scale=u_t[:, b * DP: b * DP + 1],
                    )
                    for i in range(1, DP):
                        nc.vector.scalar_tensor_tensor(
                            u2_sl,
                            w_3[:, :, i],
                            u_t[:, b * DP + i: b * DP + i + 1],
                            u2_sl,
                            OP.mult,
                            OP.add,
                        )
                    # s accumulation for this half-chain (fp32, PE idle;
                    # column-split groups give finer start dependencies)
                    nc.tensor.matmul(
                        s_ps_b[b][:],
                        ones_t[:],
                        u2_sl,
                        start=(nt == 0),
                        stop=(nt == NT - 1),
                    )

        # B_prior loads only matter in phase 2 - keep them off the W/u stream
        for nt in range(NT):
            nc.sync.dma_start(
                bp_all[:, nt * D:(nt + 1) * D],
                Bp[:, 0, nt * 128:(nt + 1) * 128].transpose([1, 0]),
            )

        # ---- phase 2 (pipelined per n-tile): A_sum -> softmax_d -> +B_prior
        #      -> S matmul ----
        # s copy to SBUF so GpSimd (no PSUM access) can read it (DVE: the
        # chain engine is free here and ACT's queue is backlogged)
        s_sb = persist.tile([128, FU], F32, tag="ssb")
        for b in range(BPC):
            nc.vector.tensor_copy(s_sb[:, b * FD:(b + 1) * FD], s_ps_b[b][:])

        # bf16 shadow of U2 for the S2 matmuls - cast lazily here, where ACT
        # is otherwise idle and off the phase-1 -> phase-2 critical path
        for nt in range(NT):
            nc.scalar.copy(
                u2bf_all[:, nt * FU:(nt + 1) * FU],
                u2_all[:, nt * FU:(nt + 1) * FU],
            )

        S2_ps = psum_S2.tile([NBD, FU], F32, tag="S2")
        POOL_TILES = (1, 2, 3, 4, 5, 6, 7, 8)  # TA on GpSimd for these n-tiles
        for nt in range(NT):
            u2_sl = u2_all[:, nt * FU:(nt + 1) * FU]
            a_sl = e_all[:, nt * NBD:(nt + 1) * NBD]  # staging (overwritten by exp)
            ta = tapool.tile([128, FU], F32, tag="ta")
            if nt in POOL_TILES:
                nc.gpsimd.tensor_tensor(ta[:], u2_sl, s_sb[:], OP.mult)
            else:
                nc.vector.tensor_tensor(ta[:], u2_sl, s_sb[:], OP.mult)
            nc.vector.tensor_reduce(
                a_sl,
                ta[:].rearrange("p (g j) -> p g j", g=NBD, j=DD),
                AX.X,
                OP.add,
            )
            # E = exp(A / sqrt(dp))
            nc.scalar.activation(a_sl, a_sl, ACTF.Exp, scale=INV_SQRT_DP)
            # z[(b)] = sum_d E ; zr = 1/z
            z_sl = z_all[:, nt * BPC:(nt + 1) * BPC]
            zr_sl = zr_all[:, nt * BPC:(nt + 1) * BPC]
            nc.vector.tensor_reduce(
                z_sl,
                a_sl.rearrange("p (b d) -> p b d", b=BPC, d=D),
                AX.X,
                OP.add,
            )
            nc.vector.reciprocal(zr_sl, z_sl)
            # cb = E * zr + B_prior, written directly as bf16 for the matmul
            cbbf_sl = cbbf_all[:, nt * NBD:(nt + 1) * NBD]
            for b in range(BPC):
                nc.vector.scalar_tensor_tensor(
                    cbbf_sl[:, b * D:(b + 1) * D],
                    a_sl[:, b * D:(b + 1) * D],
                    zr_sl[:, b: b + 1],
                    bp_all[:, nt * D:(nt + 1) * D],
                    OP.mult,
                    OP.add,
                )
            # S2 += cb.T @ U2 (bf16 operands, fp32 PSUM accumulate)
            nc.tensor.matmul(
                S2_ps[:],
                cbbf_sl,
                u2bf_all[:, nt * FU:(nt + 1) * FU],
                start=(nt == 0),
                stop=(nt == NT - 1),
            )

        # ---- phase 3: extract diagonal (b,d)=(b',d') via iota mask ----
        iota_t = persist.tile([NBD, FU], I32, tag="iota")
        nc.gpsimd.iota(
            iota_t[:], pattern=[[1, NBD], [0, DD]], base=0, channel_multiplier=-1
        )
        mask_t = persist.tile([NBD, FU], F32, tag="mask")
        nc.vector.tensor_scalar(mask_t[:], iota_t[:], 0, None, OP.is_equal)

        sm_t = smpool.tile([NBD, FU], F32, tag="sm")
        nc.vector.tensor_tensor(sm_t[:], S2_ps[:], mask_t[:], OP.mult)
        s_diag = persist.tile([NBD, DD], F32, tag="sdiag")
        nc.vector.tensor_reduce(
            s_diag[:],
            sm_t[:].rearrange("p (g j) -> p j g", g=NBD, j=DD),
            AX.X,
            OP.add,
        )

        # ---- phase 4: squash ----
        ss_t = persist.tile([NBD, DD], F32, tag="ss")
        nrm2 = persist.tile([NBD, 1], F32, tag="nrm2")
        nc.vector.tensor_tensor(ss_t[:], s_diag[:], s_diag[:], OP.mult)
        nc.vector.tensor_reduce(nrm2[:], ss_t[:], AX.X, OP.add)
        # norm via DVE Newton sqrt (bit-hack seed + 2 iterations) - keeps the
        # Exp ACT table resident (no sqrt/exp table reload in the tail)
        nrm = persist.tile([NBD, 1], F32, tag="nrm")
        seed_i = persist.tile([NBD, 1], I32, tag="seedi")
        nc.vector.tensor_scalar(
            seed_i[:], nrm2[:].bitcast(I32), 1, None, OP.logical_shift_right
        )
        nc.vector.tensor_scalar(seed_i[:], seed_i[:], 0x1FBD1DF5, None, OP.add)
        nc.vector.tensor_copy(nrm[:], seed_i[:].bitcast(F32))
        nwr = persist.tile([NBD, 1], F32, tag="nwr")
        nwt = persist.tile([NBD, 1], F32, tag="nwt")
        for _ in range(2):
            nc.vector.reciprocal(nwr[:], nrm[:])
            nc.vector.tensor_tensor(nwt[:], nrm2[:], nwr[:], OP.mult)
            nc.vector.tensor_tensor(nrm[:], nrm[:], nwt[:], OP.add)
            nc.vector.tensor_scalar(nrm[:], nrm[:], 0.5, None, OP.mult)
        en = persist.tile([NBD, 1], F32, tag="en")
        nc.scalar.activation(en[:], nrm[:], ACTF.Exp)
        en_eps = persist.tile([NBD, 1], F32, tag="eneps")
        nc.vector.tensor_scalar(en_eps[:], en[:], EPS, None, OP.add)
        r1 = persist.tile([NBD, 1], F32, tag="r1")
        nc.vector.reciprocal(r1[:], en_eps[:])
        coef = persist.tile([NBD, 1], F32, tag="coef")
        nc.vector.tensor_scalar(coef[:], r1[:], -1.0, 1.0, OP.mult, OP.add)
        nrm_eps = persist.tile([NBD, 1], F32, tag="nrmeps")
        nc.vector.tensor_scalar(nrm_eps[:], nrm[:], EPS, None, OP.add)
        r2 = persist.tile([NBD, 1], F32, tag="r2")
        nc.vector.reciprocal(r2[:], nrm_eps[:])
        fac = persist.tile([NBD, 1], F32, tag="fac")
        nc.vector.tensor_tensor(fac[:], coef[:], r2[:], OP.mult)

        res_t = persist.tile([NBD, DD], F32, tag="res")
        nc.vector.tensor_scalar(res_t[:], s_diag[:], fac[:], None, OP.mult)

        nc.sync.dma_start(out_ap.rearrange("b d j -> (b d) j"), res_t[:])


_CACHE: dict = {}


def _get_nc():
    if "nc" not in _CACHE:
        nc = bacc.Bacc(
            "TRN2", target_bir_lowering=False, debug=False, num_devices=NCORES
        )
        pc = nc.dram_tensor("primary_caps", [BPC, N, DP], F32, kind="ExternalInput").ap()
        W = nc.dram_tensor("W", [D, N, DD, DP], F32, kind="ExternalInput").ap()
        Bp = nc.dram_tensor("B_prior", [D, 1, N], F32, kind="ExternalInput").ap()
        out = nc.dram_tensor("out", [BPC, D, DD], F32, kind="ExternalOutput").ap()
        with tile.TileContext(nc) as tc:
            _build_kernel(tc, out, pc, W, Bp)
        nc.compile()
        _CACHE["nc"] = nc
    return _CACHE["nc"]


def _run(primary_caps, W, B_prior, trace=False, **kw):
    nc = _get_nc()
    in_maps = [
        {
            "primary_caps": np.ascontiguousarray(
                primary_caps[c * BPC:(c + 1) * BPC]
            ).astype(np.float32),
            "W": np.asarray(W, dtype=np.float32),
            "B_prior": np.asarray(B_prior, dtype=np.float32),
        }
        for c in range(NCORES)
    ]
    res = run_bass_kernel_spmd(nc, in_maps, list(range(NCORES)), trace=trace, **kw)
    out = np.concatenate([res.results[c]["out"] for c in range(NCORES)], axis=0)
    return out.astype(np.float32), res


def kernel(primary_caps, W, B_prior):
    out, _ = _run(primary_caps, W, B_prior, trace=False)
    return out
